# revision 11
# baseline (speedup 1.0000x reference)
"""Gaussian duration-attention upsampler on 8 Trainium2 NeuronCores (v5).

out[b,t,:] = (sum_i w[b,i,t] * emb[b,i,:]) / (sum_i w[b,i,t] + eps) + PE[t,:]
  with w[b,i,t] = exp(-(t - c[b,i])^2 / ranges[b,i]^2), c = cumsum(dur) - dur/2.

Strategy:
  - Data-parallel over batch: 4 batches/core, SPMD, no collectives.
  - Narrow Gaussians: per 128-frame output chunk only <=31 tokens matter
    (measured span max 30 on this data).  KW=32-token windows, CW=128-frame
    chunks, NJ=32 chunks/batch.  FOUR windows pack into the 128 partitions
    (partition 32k+i = token i of window k), so every W-gen op covers 4
    windows: W-gen element count is 4x smaller than 128-token banding.
  - W-gen split across engines: half the packs compute sq on ScalarE
    (Square with per-partition scale/bias), half on the otherwise-idle
    GpSimd (tensor_scalar u = sqa*t+nsq, then u*u); ONE Exp per 4 packs
    (FD=512) on ScalarE.  All W-gen is in SBUF and runs two 4-pack groups
    ahead of the consuming matmuls so LDWEIGHTS never stalls the PE.
  - Per window one K=32 matmul [W^T][eg|1] (N=257) into its own PSUM bank
    (hardware: one matmul accumulation region per bank).  4 windows/pack
    at row-tiled tile_position=(32k,0) issue back-to-back and overlap on
    the PE array.  The ones column yields the normalizer s in column 256;
    eps enters as a sacrificial token row with sq=-ln(eps), zero
    embedding, so s already contains eps.  U tiles own all 8 PSUM banks
    (per-pack [128,4,512], bufs=2); one reciprocal [128,4] per pack.
  - A 16-matmul dummy burst at t=0 (during input DMAs) warms the PE HAM
    clock gate.
  - Postproc split to balance S and V: V-packs run scalar_tensor_tensor
    straight off PSUM (cast+normalize+PE-add in one 1x pass per window);
    S-packs run per-window scaled cast-copies on ScalarE plus one
    whole-pack bf16 PE-add (VectorE 2x tensor_tensor, or GpSimd).
  - Output: chunk frame f sits on partition f (no permutation);  two
    packs stage into one [128, 8x256] tile -> one 512KB DMA per pack
    pair on the Sync HWDGE queue (GpSimd keeps no DMA work).
"""

from collections import deque

import numpy as np
import ml_dtypes

import concourse.bacc as bacc
import concourse.mybir as mybir
import concourse.tile as tile
from concourse.bass_utils import run_bass_kernel_spmd

BF16 = ml_dtypes.bfloat16

B, T_IN, D, T_OUT = 32, 512, 256, 4096
EPS = 1e-6
N_CORES = 8
BL = B // N_CORES          # batches per core (4)
CW = 128                   # chunk width (frames)
NJ = T_OUT // CW           # chunks per batch (32)
KW = 32                    # window tokens per chunk
NPACK = BL * NJ // 4       # packs per core (32); pack = (b, 4 consecutive j)
TH = 30.0                  # exp(-30) ~ 1e-13 banding threshold
NE = D + 1                 # eg columns (256 embedding + ones)

F32 = mybir.dt.float32
BF = mybir.dt.bfloat16

N_WARM = 16                # dummy matmuls to un-throttle the PE HAM clock gate
WAHEAD = 2                 # 4-pack W-gen groups emitted ahead of their matmuls
# packs whose postproc goes ScalarE-copy + tensor_tensor PE-add (rest: V stt)
S_PACKS = frozenset(round(i * 32 / 12) for i in range(12))
G_PACKS = frozenset(sorted(S_PACKS)[::2])             # their PE-add on GpSimd
GSQ_PACKS = frozenset(range(1, NPACK, 2))             # sq computed on GpSimd

_CACHE = {}


def _pe_table():
    pos = np.arange(T_OUT, dtype=np.float32)[:, None]
    div = np.exp(np.arange(0, D, 2, dtype=np.float32) * (-np.log(10000.0) / D))
    pe = np.zeros((T_OUT, D), np.float32)
    pe[:, 0::2] = np.sin(pos * div)
    pe[:, 1::2] = np.cos(pos * div)
    return pe


def _build():
    nc = bacc.Bacc(
        "TRN2",
        target_bir_lowering=False,
        debug=False,
        enable_asserts=False,
        num_devices=N_CORES,
    )
    eg_d = nc.dram_tensor("eg", (128, NPACK * NE), BF, kind="ExternalInput")
    par_d = nc.dram_tensor("par", (128, 2 * NPACK + 128), F32, kind="ExternalInput")
    pe_d = nc.dram_tensor("pe", (128, NJ * D), BF, kind="ExternalInput")
    out_d = nc.dram_tensor("out", (BL, T_OUT, D), BF, kind="ExternalOutput")
    # frame 1024*Q + 128*k + t lives on partition t, free offset k*D + d
    outv = out_d[:].rearrange("b (Q k t) d -> b Q t k d", Q=NJ // 8, k=8, t=128)

    Sq = mybir.ActivationFunctionType.Square
    Ex = mybir.ActivationFunctionType.Exp
    Iden = mybir.ActivationFunctionType.Identity
    ADD = mybir.AluOpType.add
    MUL = mybir.AluOpType.mult

    with tile.TileContext(nc) as tc:
        with (
            tc.tile_pool(name="const", bufs=1) as cp,
            tc.tile_pool(name="u2", bufs=2) as u2p,
            tc.tile_pool(name="sq", bufs=2) as sqp,
            tc.tile_pool(name="w", bufs=3) as wp,
            tc.tile_pool(name="ub", bufs=3) as ubp,
            tc.tile_pool(name="ob", bufs=4) as obp,
            tc.tile_pool(name="r", bufs=6) as rp,
            tc.tile_pool(name="pu", bufs=2, space="PSUM") as pup,
        ):
            # dummy activation with no DMA deps: pulls the ACT-table load
            # to the head of the Scalar queue, overlapping it with input DMAs
            dmy = cp.tile([128, 8], F32)
            nc.gpsimd.memset(dmy[:], 0.0)
            zb = dmy[:, 0:1]
            dmy2 = cp.tile([128, 8], F32)
            nc.scalar.activation(dmy2[:], dmy[:], Sq, bias=zb)
            nc.scalar.activation(dmy2[:], dmy[:], Ex, bias=zb, scale=-1.0)

            # HAM warm-up: a burst of dummy back-to-back matmuls (no DMA deps)
            # runs during the input DMAs and un-throttles the PE clock gate
            wdm = cp.tile([128, 128], BF)
            edm = cp.tile([128, 512], BF)
            nc.gpsimd.memset(wdm[:], 0.25)
            nc.gpsimd.memset(edm[:], 0.25)
            for i in range(N_WARM):
                wps = pup.tile([128, 4, 512], F32, name=f"warm{i}", tag="u")
                nc.tensor.matmul(wps[:, 0, :], wdm[:], edm[:], start=True, stop=True)

            # par: [sqa (NPACK) | nsqac (NPACK) | iota (128)] per partition
            par_sb = cp.tile([128, 2 * NPACK + 128], F32)
            eg_sbs = [cp.tile([128, 8 * NE], BF, name=f"eg{b}") for b in range(BL)]
            pe_sbs = [cp.tile([128, 8 * D], BF, name=f"pe{q}") for q in range(4)]
            nc.sync.dma_start(par_sb[:], par_d[:])
            nc.sync.dma_start(eg_sbs[0][:], eg_d[:, 0 : 8 * NE])
            nc.sync.dma_start(pe_sbs[0][:], pe_d[:, 0 : 8 * D])
            nc.sync.dma_start(pe_sbs[1][:], pe_d[:, 8 * D : 16 * D])
            nc.sync.dma_start(eg_sbs[1][:], eg_d[:, 8 * NE : 16 * NE])
            nc.sync.dma_start(pe_sbs[2][:], pe_d[:, 16 * D : 24 * D])
            nc.sync.dma_start(pe_sbs[3][:], pe_d[:, 24 * D : 32 * D])
            nc.sync.dma_start(eg_sbs[2][:], eg_d[:, 16 * NE : 24 * NE])
            nc.sync.dma_start(eg_sbs[3][:], eg_d[:, 24 * NE : 32 * NE])
            sqa_sb = par_sb[:, 0:NPACK]
            nsq_sb = par_sb[:, NPACK : 2 * NPACK]
            iota_sb = par_sb[:, 2 * NPACK :]

            obs = {}
            ubs = {}

            def emit_wgen(gr):
                sqt = sqp.tile([128, 4, 128], F32, name=f"sq{gr}", tag="sq")
                for j in range(4):
                    p = 4 * gr + j
                    if p in GSQ_PACKS:
                        ut = u2p.tile([128, 128], F32, name=f"u2{p}", tag="u2")
                        nc.gpsimd.tensor_scalar(
                            ut[:], iota_sb,
                            sqa_sb[:, p : p + 1], nsq_sb[:, p : p + 1],
                            MUL, ADD,
                        )
                        nc.gpsimd.tensor_tensor(sqt[:, j, :], ut[:], ut[:], MUL)
                    else:
                        nc.scalar.activation(
                            sqt[:, j, :], iota_sb, Sq,
                            bias=nsq_sb[:, p : p + 1],
                            scale=sqa_sb[:, p : p + 1],
                        )
                wg4 = wp.tile([128, 4, 128], BF, name=f"w{gr}", tag="w")
                nc.scalar.activation(wg4[:], sqt[:], Ex, bias=zb, scale=-1.0)
                return wg4

            def emit_post(st):
                p, ups, r4 = st
                b, pp = divmod(p, NJ // 4)
                pe_t = pe_sbs[pp // 2]
                po = (pp % 2) * 4 * D
                oo = (p % 2) * 4 * D
                if p % 2 == 0:
                    obs[p] = obp.tile([128, 8 * D], BF, name=f"ob{p}", tag="ob")
                    ob = obs[p]
                else:
                    ob = obs.pop(p - 1)
                if p not in S_PACKS:
                    # V-direct path: stt straight off PSUM does cast +
                    # normalize + PE-add in one 1x pass per window
                    for k in range(4):
                        nc.vector.scalar_tensor_tensor(
                            ob[:, oo + k * D : oo + (k + 1) * D],
                            ups[:, k, 0:D],
                            r4[:, k : k + 1],
                            pe_t[:, po + k * D : po + (k + 1) * D],
                            MUL,
                            ADD,
                        )
                else:
                    # S path: per-window scaled cast-copy on ScalarE, then one
                    # whole-pack bf16 PE-add on VectorE (2x) or GpSimd
                    ub = ubp.tile([128, 4 * D], BF, name=f"ub{p}", tag="ub")
                    for k in range(4):
                        nc.scalar.activation(
                            ub[:, k * D : (k + 1) * D],
                            ups[:, k, 0:D],
                            Iden,
                            bias=zb,
                            scale=r4[:, k : k + 1],
                        )
                    eng = nc.gpsimd if p in G_PACKS else nc.vector
                    eng.tensor_tensor(
                        ob[:, oo : oo + 4 * D], ub[:], pe_t[:, po : po + 4 * D], ADD
                    )
                if p % 2 == 1:
                    nc.sync.dma_start(
                        outv[b, pp // 2], ob[:].rearrange("t (k d) -> t k d", k=8)
                    )

            pending = deque()
            wgs = {}
            for gr in range(WAHEAD):
                wgs[gr] = emit_wgen(gr)
            for gr in range(NPACK // 4):
                if gr + WAHEAD < NPACK // 4:
                    wgs[gr + WAHEAD] = emit_wgen(gr + WAHEAD)
                wg4 = wgs.pop(gr)
                for j in range(4):
                    p = 4 * gr + j
                    b, pp = divmod(p, NJ // 4)
                    ups = pup.tile([128, 4, 512], F32, name=f"u{p}", tag="u")
                    for k in range(4):
                        nc.tensor.matmul(
                            ups[:, k, 0:NE],
                            wg4[32 * k : 32 * k + 32, j, :],
                            eg_sbs[b][32 * k : 32 * k + 32, pp * NE : (pp + 1) * NE],
                            start=True,
                            stop=True,
                            tile_position=(32 * k, 0),
                        )
                    r4 = rp.tile([128, 4], F32, name=f"r{p}", tag="r")
                    nc.vector.reciprocal(r4[:], ups[:, :, 256])
                    pending.append((p, ups, r4))
                    while len(pending) > 1:
                        emit_post(pending.popleft())
            while pending:
                emit_post(pending.popleft())

    nc.compile()
    return nc


def kernel(embeddings, durations, ranges, t_out):
    assert int(t_out) == T_OUT
    emb = np.asarray(embeddings, dtype=np.float32)
    dur = np.asarray(durations, dtype=np.float32)[:, :, 0]
    rng = np.asarray(ranges, dtype=np.float32)[:, :, 0]

    # ---- host preprocessing: O(B*T_in) scalars + window selection ----
    c = np.cumsum(dur, axis=1, dtype=np.float32) - 0.5 * dur   # (B, T_IN)
    a = rng.astype(np.float32) ** -2
    reach = np.sqrt(TH) * rng

    lo_r, hi_r = c - reach, c + reach
    starts = np.zeros((B, NJ), np.int32)
    for b in range(B):
        for j in range(NJ):
            qual = np.nonzero((lo_r[b] <= CW * j + CW - 1) & (hi_r[b] >= CW * j))[0]
            if len(qual):
                assert qual[-1] - qual[0] + 1 <= KW - 1, "window overflow"
                starts[b, j] = qual[0]
    starts = np.minimum(starts, T_IN - (KW - 1))
    # coverage assert (windows are contiguous token ranges)
    for b in range(B):
        for j in range(NJ):
            qual = np.nonzero((lo_r[b] <= CW * j + CW - 1) & (hi_r[b] >= CW * j))[0]
            if len(qual):
                assert starts[b, j] <= qual[0] and qual[-1] < starts[b, j] + KW - 1

    kidx = starts[:, :, None] + np.arange(KW)[None, None, :]   # (B, NJ, KW)
    kidx = np.minimum(kidx, T_IN - 1)
    bidx = np.arange(B)[:, None, None]
    cg = c[bidx, kidx]
    ag = a[bidx, kidx]
    center = (np.arange(NJ, dtype=np.float32) * CW + CW / 2)[None, :, None]
    cc = cg - center
    sqa = np.sqrt(ag)
    nsq = -sqa * cc
    # sacrificial eps token: sq = -ln(eps) const -> w = eps for all t
    sqa[:, :, KW - 1] = 0.0
    nsq[:, :, KW - 1] = np.float32(np.sqrt(-np.log(EPS)))

    egg = np.ones((B, NJ, KW, NE), BF16)
    egg[:, :, :, 0:D] = emb[bidx, kidx].astype(BF16)            # (B, NJ, KW, D)
    egg[:, :, KW - 1, 0:D] = 0

    iota = np.broadcast_to(
        np.arange(128, dtype=np.float32) - 64.0, (128, 128)
    ).copy()
    pe = _pe_table().reshape(NJ, 128, D).transpose(1, 0, 2).reshape(128, NJ * D)
    pe = pe.astype(BF16)

    if 0 not in _CACHE:
        _CACHE[0] = _build()
    nc = _CACHE[0]

    in_maps = []
    for i in range(N_CORES):
        bs = slice(i * BL, (i + 1) * BL)
        # eg: partition 32k+i <- token i of window k; col (b*8+pp)*NE + e
        eg5 = egg[bs].reshape(BL, NJ // 4, 4, KW, NE)           # (b, pp, k, i, e)
        eg_core = np.ascontiguousarray(
            eg5.transpose(2, 3, 0, 1, 4).reshape(4 * KW, NPACK * NE)
        )
        # par: [sqa | nsqac | iota]; col p, partition 32k+i
        sqa5 = sqa[bs].reshape(BL, NJ // 4, 4, KW)
        sqa_core = sqa5.transpose(2, 3, 0, 1).reshape(4 * KW, NPACK)
        nsq5 = nsq[bs].reshape(BL, NJ // 4, 4, KW)
        nsq_core = nsq5.transpose(2, 3, 0, 1).reshape(4 * KW, NPACK)
        par_core = np.ascontiguousarray(
            np.concatenate([sqa_core, nsq_core, iota], axis=1).astype(np.float32)
        )
        in_maps.append({
            "eg": eg_core,
            "par": par_core,
            "pe": pe,
        })

    res = run_bass_kernel_spmd(nc, in_maps, core_ids=list(range(N_CORES)))
    out = np.concatenate([r["out"] for r in res.results], axis=0)
    return out.astype(np.float32)


# revision 13
# speedup vs baseline: 1.0057x; 1.0057x over previous
"""Gaussian duration-attention upsampler on 8 Trainium2 NeuronCores (v5).

out[b,t,:] = (sum_i w[b,i,t] * emb[b,i,:]) / (sum_i w[b,i,t] + eps) + PE[t,:]
  with w[b,i,t] = exp(-(t - c[b,i])^2 / ranges[b,i]^2), c = cumsum(dur) - dur/2.

Strategy:
  - Data-parallel over batch: 4 batches/core, SPMD, no collectives.
  - Narrow Gaussians: per 128-frame output chunk only <=31 tokens matter
    (measured span max 30 on this data).  KW=32-token windows, CW=128-frame
    chunks, NJ=32 chunks/batch.  FOUR windows pack into the 128 partitions
    (partition 32k+i = token i of window k), so every W-gen op covers 4
    windows: W-gen element count is 4x smaller than 128-token banding.
  - W-gen split across engines: half the packs compute sq on ScalarE
    (Square with per-partition scale/bias), half on the otherwise-idle
    GpSimd (tensor_scalar u = sqa*t+nsq, then u*u); ONE Exp per 4 packs
    (FD=512) on ScalarE.  All W-gen is in SBUF and runs two 4-pack groups
    ahead of the consuming matmuls so LDWEIGHTS never stalls the PE.
  - Per window one K=32 matmul [W^T][eg|1] (N=257) into its own PSUM bank
    (hardware: one matmul accumulation region per bank).  4 windows/pack
    at row-tiled tile_position=(32k,0) issue back-to-back and overlap on
    the PE array.  The ones column yields the normalizer s in column 256;
    eps enters as a sacrificial token row with sq=-ln(eps), zero
    embedding, so s already contains eps.  U tiles own all 8 PSUM banks
    (per-pack [128,4,512], bufs=2); one reciprocal [128,4] per pack.
  - A 16-matmul dummy burst at t=0 (during input DMAs) warms the PE HAM
    clock gate.
  - Postproc split to balance S and V: V-packs run scalar_tensor_tensor
    straight off PSUM (cast+normalize+PE-add in one 1x pass per window);
    S-packs run per-window scaled cast-copies on ScalarE plus one
    whole-pack bf16 PE-add (VectorE 2x tensor_tensor, or GpSimd).
  - Output: chunk frame f sits on partition f (no permutation);  two
    packs stage into one [128, 8x256] tile -> one 512KB DMA per pack
    pair on the Sync HWDGE queue (GpSimd keeps no DMA work).
"""

from collections import deque

import numpy as np
import ml_dtypes

import concourse.bacc as bacc
import concourse.mybir as mybir
import concourse.tile as tile
from concourse.bass_utils import run_bass_kernel_spmd

BF16 = ml_dtypes.bfloat16

B, T_IN, D, T_OUT = 32, 512, 256, 4096
EPS = 1e-6
N_CORES = 8
BL = B // N_CORES          # batches per core (4)
CW = 128                   # chunk width (frames)
NJ = T_OUT // CW           # chunks per batch (32)
KW = 32                    # window tokens per chunk
NPACK = BL * NJ // 4       # packs per core (32); pack = (b, 4 consecutive j)
TH = 30.0                  # exp(-30) ~ 1e-13 banding threshold
NE = D + 1                 # eg columns (256 embedding + ones)

F32 = mybir.dt.float32
BF = mybir.dt.bfloat16

N_WARM = 16                # dummy matmuls to un-throttle the PE HAM clock gate
WAHEAD = 2                 # 4-pack W-gen groups emitted ahead of their matmuls
# packs whose postproc goes ScalarE-copy + tensor_tensor PE-add (rest: V stt)
S_PACKS = frozenset(round(i * 32 / 14) for i in range(14))
G_PACKS = frozenset(sorted(S_PACKS)[::2])             # their PE-add on GpSimd
GSQ_PACKS = frozenset(range(1, NPACK, 2))             # sq computed on GpSimd

_CACHE = {}


def _pe_table():
    pos = np.arange(T_OUT, dtype=np.float32)[:, None]
    div = np.exp(np.arange(0, D, 2, dtype=np.float32) * (-np.log(10000.0) / D))
    pe = np.zeros((T_OUT, D), np.float32)
    pe[:, 0::2] = np.sin(pos * div)
    pe[:, 1::2] = np.cos(pos * div)
    return pe


def _build():
    nc = bacc.Bacc(
        "TRN2",
        target_bir_lowering=False,
        debug=False,
        enable_asserts=False,
        num_devices=N_CORES,
    )
    eg_d = nc.dram_tensor("eg", (128, NPACK * NE), BF, kind="ExternalInput")
    par_d = nc.dram_tensor("par", (128, 2 * NPACK + 128), F32, kind="ExternalInput")
    pe_d = nc.dram_tensor("pe", (128, NJ * D), BF, kind="ExternalInput")
    out_d = nc.dram_tensor("out", (BL, T_OUT, D), BF, kind="ExternalOutput")
    # frame 1024*Q + 128*k + t lives on partition t, free offset k*D + d
    outv = out_d[:].rearrange("b (Q k t) d -> b Q t k d", Q=NJ // 8, k=8, t=128)

    Sq = mybir.ActivationFunctionType.Square
    Ex = mybir.ActivationFunctionType.Exp
    Iden = mybir.ActivationFunctionType.Identity
    ADD = mybir.AluOpType.add
    MUL = mybir.AluOpType.mult

    with tile.TileContext(nc) as tc:
        with (
            tc.tile_pool(name="const", bufs=1) as cp,
            tc.tile_pool(name="u2", bufs=2) as u2p,
            tc.tile_pool(name="sq", bufs=2) as sqp,
            tc.tile_pool(name="w", bufs=3) as wp,
            tc.tile_pool(name="ub", bufs=3) as ubp,
            tc.tile_pool(name="ob", bufs=4) as obp,
            tc.tile_pool(name="r", bufs=6) as rp,
            tc.tile_pool(name="pu", bufs=2, space="PSUM") as pup,
        ):
            # dummy activation with no DMA deps: pulls the ACT-table load
            # to the head of the Scalar queue, overlapping it with input DMAs
            dmy = cp.tile([128, 8], F32)
            nc.gpsimd.memset(dmy[:], 0.0)
            zb = dmy[:, 0:1]
            dmy2 = cp.tile([128, 8], F32)
            nc.scalar.activation(dmy2[:], dmy[:], Sq, bias=zb)
            nc.scalar.activation(dmy2[:], dmy[:], Ex, bias=zb, scale=-1.0)

            # HAM warm-up: a burst of dummy back-to-back matmuls (no DMA deps)
            # runs during the input DMAs and un-throttles the PE clock gate
            wdm = cp.tile([128, 128], BF)
            edm = cp.tile([128, 512], BF)
            nc.gpsimd.memset(wdm[:], 0.25)
            nc.gpsimd.memset(edm[:], 0.25)
            for i in range(N_WARM):
                wps = pup.tile([128, 4, 512], F32, name=f"warm{i}", tag="u")
                nc.tensor.matmul(wps[:, 0, :], wdm[:], edm[:], start=True, stop=True)

            # par: [sqa (NPACK) | nsqac (NPACK) | iota (128)] per partition
            par_sb = cp.tile([128, 2 * NPACK + 128], F32)
            eg_sbs = [cp.tile([128, 8 * NE], BF, name=f"eg{b}") for b in range(BL)]
            pe_sbs = [cp.tile([128, 8 * D], BF, name=f"pe{q}") for q in range(4)]
            nc.sync.dma_start(par_sb[:], par_d[:])
            nc.sync.dma_start(eg_sbs[0][:], eg_d[:, 0 : 8 * NE])
            nc.sync.dma_start(pe_sbs[0][:], pe_d[:, 0 : 8 * D])
            nc.sync.dma_start(pe_sbs[1][:], pe_d[:, 8 * D : 16 * D])
            nc.sync.dma_start(eg_sbs[1][:], eg_d[:, 8 * NE : 16 * NE])
            nc.sync.dma_start(pe_sbs[2][:], pe_d[:, 16 * D : 24 * D])
            nc.sync.dma_start(pe_sbs[3][:], pe_d[:, 24 * D : 32 * D])
            nc.sync.dma_start(eg_sbs[2][:], eg_d[:, 16 * NE : 24 * NE])
            nc.sync.dma_start(eg_sbs[3][:], eg_d[:, 24 * NE : 32 * NE])
            sqa_sb = par_sb[:, 0:NPACK]
            nsq_sb = par_sb[:, NPACK : 2 * NPACK]
            iota_sb = par_sb[:, 2 * NPACK :]

            obs = {}
            ubs = {}

            def emit_wgen(gr):
                sqt = sqp.tile([128, 4, 128], F32, name=f"sq{gr}", tag="sq")
                for j in range(4):
                    p = 4 * gr + j
                    if p in GSQ_PACKS:
                        ut = u2p.tile([128, 128], F32, name=f"u2{p}", tag="u2")
                        nc.gpsimd.tensor_scalar(
                            ut[:], iota_sb,
                            sqa_sb[:, p : p + 1], nsq_sb[:, p : p + 1],
                            MUL, ADD,
                        )
                        nc.gpsimd.tensor_tensor(sqt[:, j, :], ut[:], ut[:], MUL)
                    else:
                        nc.scalar.activation(
                            sqt[:, j, :], iota_sb, Sq,
                            bias=nsq_sb[:, p : p + 1],
                            scale=sqa_sb[:, p : p + 1],
                        )
                wg4 = wp.tile([128, 4, 128], BF, name=f"w{gr}", tag="w")
                nc.scalar.activation(wg4[:], sqt[:], Ex, bias=zb, scale=-1.0)
                return wg4

            def emit_post(st):
                p, ups, r4 = st
                b, pp = divmod(p, NJ // 4)
                pe_t = pe_sbs[pp // 2]
                po = (pp % 2) * 4 * D
                oo = (p % 2) * 4 * D
                if p % 2 == 0:
                    obs[p] = obp.tile([128, 8 * D], BF, name=f"ob{p}", tag="ob")
                    ob = obs[p]
                else:
                    ob = obs.pop(p - 1)
                if p not in S_PACKS:
                    # V-direct path: stt straight off PSUM does cast +
                    # normalize + PE-add in one 1x pass per window
                    for k in range(4):
                        nc.vector.scalar_tensor_tensor(
                            ob[:, oo + k * D : oo + (k + 1) * D],
                            ups[:, k, 0:D],
                            r4[:, k : k + 1],
                            pe_t[:, po + k * D : po + (k + 1) * D],
                            MUL,
                            ADD,
                        )
                else:
                    # S path: per-window scaled cast-copy on ScalarE, then one
                    # whole-pack bf16 PE-add on VectorE (2x) or GpSimd
                    ub = ubp.tile([128, 4 * D], BF, name=f"ub{p}", tag="ub")
                    for k in range(4):
                        nc.scalar.activation(
                            ub[:, k * D : (k + 1) * D],
                            ups[:, k, 0:D],
                            Iden,
                            bias=zb,
                            scale=r4[:, k : k + 1],
                        )
                    eng = nc.gpsimd if p in G_PACKS else nc.vector
                    eng.tensor_tensor(
                        ob[:, oo : oo + 4 * D], ub[:], pe_t[:, po : po + 4 * D], ADD
                    )
                if p % 2 == 1:
                    nc.sync.dma_start(
                        outv[b, pp // 2], ob[:].rearrange("t (k d) -> t k d", k=8)
                    )

            pending = deque()
            wgs = {}
            for gr in range(WAHEAD):
                wgs[gr] = emit_wgen(gr)
            for gr in range(NPACK // 4):
                if gr + WAHEAD < NPACK // 4:
                    wgs[gr + WAHEAD] = emit_wgen(gr + WAHEAD)
                wg4 = wgs.pop(gr)
                for j in range(4):
                    p = 4 * gr + j
                    b, pp = divmod(p, NJ // 4)
                    ups = pup.tile([128, 4, 512], F32, name=f"u{p}", tag="u")
                    for k in range(4):
                        nc.tensor.matmul(
                            ups[:, k, 0:NE],
                            wg4[32 * k : 32 * k + 32, j, :],
                            eg_sbs[b][32 * k : 32 * k + 32, pp * NE : (pp + 1) * NE],
                            start=True,
                            stop=True,
                            tile_position=(32 * k, 0),
                        )
                    # the lagged postproc is emitted BEFORE this pack's recip:
                    # VectorE then has a full pack of queued stt work to chew
                    # on while the recip's matmul dependency completes, instead
                    # of stalling its in-order queue on a TensorE round-trip
                    while len(pending) > 0:
                        emit_post(pending.popleft())
                    r4 = rp.tile([128, 4], F32, name=f"r{p}", tag="r")
                    nc.vector.reciprocal(r4[:], ups[:, :, 256])
                    pending.append((p, ups, r4))
            while pending:
                emit_post(pending.popleft())

    nc.compile()
    return nc


def kernel(embeddings, durations, ranges, t_out):
    assert int(t_out) == T_OUT
    emb = np.asarray(embeddings, dtype=np.float32)
    dur = np.asarray(durations, dtype=np.float32)[:, :, 0]
    rng = np.asarray(ranges, dtype=np.float32)[:, :, 0]

    # ---- host preprocessing: O(B*T_in) scalars + window selection ----
    c = np.cumsum(dur, axis=1, dtype=np.float32) - 0.5 * dur   # (B, T_IN)
    a = rng.astype(np.float32) ** -2
    reach = np.sqrt(TH) * rng

    lo_r, hi_r = c - reach, c + reach
    starts = np.zeros((B, NJ), np.int32)
    for b in range(B):
        for j in range(NJ):
            qual = np.nonzero((lo_r[b] <= CW * j + CW - 1) & (hi_r[b] >= CW * j))[0]
            if len(qual):
                assert qual[-1] - qual[0] + 1 <= KW - 1, "window overflow"
                starts[b, j] = qual[0]
    starts = np.minimum(starts, T_IN - (KW - 1))
    # coverage assert (windows are contiguous token ranges)
    for b in range(B):
        for j in range(NJ):
            qual = np.nonzero((lo_r[b] <= CW * j + CW - 1) & (hi_r[b] >= CW * j))[0]
            if len(qual):
                assert starts[b, j] <= qual[0] and qual[-1] < starts[b, j] + KW - 1

    kidx = starts[:, :, None] + np.arange(KW)[None, None, :]   # (B, NJ, KW)
    kidx = np.minimum(kidx, T_IN - 1)
    bidx = np.arange(B)[:, None, None]
    cg = c[bidx, kidx]
    ag = a[bidx, kidx]
    center = (np.arange(NJ, dtype=np.float32) * CW + CW / 2)[None, :, None]
    cc = cg - center
    sqa = np.sqrt(ag)
    nsq = -sqa * cc
    # sacrificial eps token: sq = -ln(eps) const -> w = eps for all t
    sqa[:, :, KW - 1] = 0.0
    nsq[:, :, KW - 1] = np.float32(np.sqrt(-np.log(EPS)))

    egg = np.ones((B, NJ, KW, NE), BF16)
    egg[:, :, :, 0:D] = emb[bidx, kidx].astype(BF16)            # (B, NJ, KW, D)
    egg[:, :, KW - 1, 0:D] = 0

    iota = np.broadcast_to(
        np.arange(128, dtype=np.float32) - 64.0, (128, 128)
    ).copy()
    pe = _pe_table().reshape(NJ, 128, D).transpose(1, 0, 2).reshape(128, NJ * D)
    pe = pe.astype(BF16)

    if 0 not in _CACHE:
        _CACHE[0] = _build()
    nc = _CACHE[0]

    in_maps = []
    for i in range(N_CORES):
        bs = slice(i * BL, (i + 1) * BL)
        # eg: partition 32k+i <- token i of window k; col (b*8+pp)*NE + e
        eg5 = egg[bs].reshape(BL, NJ // 4, 4, KW, NE)           # (b, pp, k, i, e)
        eg_core = np.ascontiguousarray(
            eg5.transpose(2, 3, 0, 1, 4).reshape(4 * KW, NPACK * NE)
        )
        # par: [sqa | nsqac | iota]; col p, partition 32k+i
        sqa5 = sqa[bs].reshape(BL, NJ // 4, 4, KW)
        sqa_core = sqa5.transpose(2, 3, 0, 1).reshape(4 * KW, NPACK)
        nsq5 = nsq[bs].reshape(BL, NJ // 4, 4, KW)
        nsq_core = nsq5.transpose(2, 3, 0, 1).reshape(4 * KW, NPACK)
        par_core = np.ascontiguousarray(
            np.concatenate([sqa_core, nsq_core, iota], axis=1).astype(np.float32)
        )
        in_maps.append({
            "eg": eg_core,
            "par": par_core,
            "pe": pe,
        })

    res = run_bass_kernel_spmd(nc, in_maps, core_ids=list(range(N_CORES)))
    out = np.concatenate([r["out"] for r in res.results], axis=0)
    return out.astype(np.float32)
